# revision 26
# baseline (speedup 1.0000x reference)
"""Distributed causal attention with RoPE for trn2 (8 NeuronCores).

Problem: B=2, S=2048, DIM=2048, H=16 heads, D=128.
  out = softmax(causal(rope(xq) @ rope(xk)^T / sqrt(D))) @ xv @ wo^T

Sharding: tensor-parallel over heads, 8-way: each core owns 2 global heads
for BOTH batches (4 local attention instances).  Attention is fully local;
the only collective is an 8-rank AllToAll per sequence chunk, which leaves
every core with the full per-batch attention output in global-head-major
row order (identical static indices on every core -> clean SPMD).  Each
core then computes one (batch, 512-column) slice of the output projection.

The whole kernel is one software pipeline over the 4 sequence chunks:
  proj(chunk t) -> attention(t) -> AllToAll(t) -> out-projection(t-2)
with the out-projection tiles interleaved between attention instances so
the PE array has independent work whenever the ACT-exp stream stalls the
scores->reduce chain.  The last chunk's attention + AllToAll are split
into two 256-column halves so the final collective's latency hides behind
the second half's attention and the trailing out-projections.

Layout strategy (on-device matmuls contract over the partition axis):
  - host feeds x^T and w^T so no on-device transposes are needed
  - Q,K are produced transposed ([d, s]); RoPE pair-swap runs on the
    vector engine via stream_shuffle straight out of PSUM
  - softmax runs on transposed scores, flash-style streamed over k-tiles:
    exp on ACT, causal masking by multiply.  For small k-tile counts the
    row-sum (a partition reduction) accumulates through a bf16
    ones-matmul per k-tile; for larger counts the exp tiles are summed on
    the vector engine (f32) and a single fp32 ones-matmul per
    (instance, q-block) does the partition reduction, keeping the tensor
    engine stream ~2 matmuls per k-tile instead of 3
  - row-sum reciprocal via the fast custom-DVE Newton iteration
  - compute dtype bf16 (fp32 PSUM accumulation), output fp32
"""

import math
import sys

sys.path.insert(0, "/opt/trn_rl_repo")

import numpy as np
import ml_dtypes

import concourse.bass as bass
import concourse.mybir as mybir
import concourse.tile as tile
from concourse import bacc
from concourse.bass_utils import run_bass_kernel_spmd

BF16 = mybir.dt.bfloat16
F32 = mybir.dt.float32

B, S, DIM, H, D = 2, 2048, 2048, 16, 128
NCORES = 8
HPC = 2                  # global heads per core
NI = B * HPC             # local attention instances (batch x head) = 4
HD = HPC * D             # local hidden slice = 256
OSL = 512                # output column slice per core
GROUP = [list(range(NCORES))]
INV_SQRT_D = 1.0 / math.sqrt(D)
SWAP_MASK = [i + 1 if i % 2 == 0 else i - 1 for i in range(32)]

SBW = 512                # s-block width (= pipeline chunk width)
NSB = S // SBW           # 4
NKT = DIM // 128         # 16 contraction tiles for projections

LAST_RESULT = None
_CACHED_NC = None


def _proj_block(nc, b, sb, xt, wq_t, wk_t, wv_t, cos_t, sin_t,
                qTc, kT_t, v_t, ta, psP):
    """QKV projections (+RoPE on Q,K) for (batch b, s-block sb).

    Q^T goes to the per-chunk tile qTc [128, NI, 512]; K^T to the resident
    kT_t at the chunk's s-range.
    """
    ssl = slice(sb * SBW, (sb + 1) * SBW)
    for wi, (w_t, qk) in enumerate(((wq_t, 0), (wk_t, 1))):
        for j in range(HPC):
            hi = b * HPC + j
            pp = psP.tile([128, SBW], F32, tag="pj", bufs=3,
                          name=f"pp{b}{sb}{j}{wi}")
            for i in range(NKT):
                nc.tensor.matmul(
                    pp[:],
                    lhsT=w_t[:, i, j * 128:(j + 1) * 128],
                    rhs=xt[:, i, :],
                    start=(i == 0), stop=(i == NKT - 1))
            # rope: out = x*cos + pair_swap(x)*sin_signed
            dst = qTc[:, hi, :] if qk == 0 else kT_t[:, hi, ssl]
            swq = ta.tile([128, SBW], F32, tag="swq", bufs=2,
                          name=f"swq{b}{sb}{j}{wi}")
            nc.vector.stream_shuffle(swq[:], pp[:], SWAP_MASK)
            nc.vector.tensor_tensor(
                out=dst, in0=pp[:], in1=cos_t[:, ssl],
                op=mybir.AluOpType.mult)
            m2 = ta.tile([128, SBW], BF16, tag="m2", bufs=2,
                         name=f"m2_{b}{sb}{j}{wi}")
            nc.vector.tensor_tensor(
                out=m2[:], in0=swq[:], in1=sin_t[:, ssl],
                op=mybir.AluOpType.mult)
            nc.vector.tensor_tensor(
                out=dst, in0=dst, in1=m2[:], op=mybir.AluOpType.add)
    # V in natural layout [s, dv]
    for m in range(4):
        pv = psP.tile([128, HD], F32, tag="pj", bufs=3, name=f"pv{b}{sb}{m}")
        for i in range(NKT):
            nc.tensor.matmul(
                pv[:],
                lhsT=xt[:, i, m * 128:(m + 1) * 128],
                rhs=wv_t[:, i, :],
                start=(i == 0), stop=(i == NKT - 1))
        nc.scalar.copy(v_t[:, b * 16 + 4 * sb + m, :], pv[:])


def _att_block(nc, qb, hi, qoff, qw, qTc, kT_t, v_t, ao_i, msk_t,
               ones_bf, ones_f32, eb, tb, psP, half_tag=""):
    """Flash-style attention for one (q-block, instance), q columns
    [qoff, qoff+qw) within the chunk.

    Streams k-tiles: scores -> exp -> mask.  The PV product is staggered
    two k-tiles behind the scores matmul to hide the ACT exp latency.
    The row-sum (partition reduction of the exp tiles) either accumulates
    through per-k-tile bf16 ones-matmuls (small nkt) or through f32 adds
    on the vector engine with a single fp32 ones-matmul at the end.
    """
    b, j = hi // HPC, hi % HPC
    # number of k-tiles covering q < qb*512 + qoff + qw
    nkt = (qb * SBW + qoff + qw + 127) // 128
    es_route = nkt >= 8

    pr = psP.tile([128, 512], F32, tag="sc", bufs=3,
                  name=f"pr{half_tag}{qb}{hi}")
    pu = psP.tile([128, 512], F32, tag="pu", bufs=2,
                  name=f"pu{half_tag}{qb}{hi}")
    es = eb.tile([128, 512], F32, tag="es", bufs=2,
                 name=f"es{half_tag}{qb}{hi}")

    def scores(kt):
        dj = kt - 4 * qb
        # c0: first valid column (chunk-relative) for diagonal tiles,
        # clipped to this q-slice
        c0 = min(max(max(dj, 0) * 128 - qoff, 0), qw)
        pk = psP.tile([128, 512], F32, tag="sc", bufs=3,
                      name=f"pk{half_tag}{qb}{hi}{kt}")
        nc.tensor.matmul(
            pk[:, c0:qw],
            lhsT=kT_t[:, hi, kt * 128:(kt + 1) * 128],
            rhs=qTc[:, hi, qoff + c0:qoff + qw],
            start=True, stop=True)
        et = eb.tile([128, 512], BF16, tag="e", bufs=4,
                     name=f"et{half_tag}{qb}{hi}{kt}")
        nc.scalar.activation(
            et[:, c0:qw], pk[:, c0:qw], mybir.ActivationFunctionType.Exp,
            scale=INV_SQRT_D)
        if dj >= 0 and (dj + 1) * 128 > qoff + c0:
            # diagonal crosses this q-slice: zero the masked positions
            nc.vector.tensor_tensor(
                out=et[:, c0:qw], in0=et[:, c0:qw],
                in1=msk_t[:, dj, qoff + c0:qoff + qw],
                op=mybir.AluOpType.mult)
        if es_route:
            if kt == 0:
                nc.vector.tensor_copy(es[:, :qw], et[:, :qw])
            else:
                nc.vector.tensor_tensor(
                    out=es[:, c0:qw], in0=es[:, c0:qw], in1=et[:, c0:qw],
                    op=mybir.AluOpType.add)
        return et, c0

    def reduce(kt, et, c0):
        if not es_route:
            nc.tensor.matmul(pr[:, c0:qw], lhsT=ones_bf[:], rhs=et[:, c0:qw],
                             start=(kt == 0), stop=(kt == nkt - 1))
        nc.tensor.matmul(
            pu[:, c0:qw],
            lhsT=v_t[:, b * 16 + kt, j * 128:(j + 1) * 128],
            rhs=et[:, c0:qw],
            start=(kt == 0), stop=(kt == nkt - 1))

    pend = []
    for kt in range(nkt):
        pend.append((kt, scores(kt)))
        if len(pend) > 2:
            k0, (et0, c00) = pend.pop(0)
            reduce(k0, et0, c00)
    for k0, (et0, c00) in pend:
        reduce(k0, et0, c00)

    if es_route:
        # one bf16 cast + bf16 ones-matmul is ~3x cheaper on the PE than an
        # fp32 matmul (4 cycles/row); the single rounding of the final sum
        # costs <0.2% on the denominator
        esb = eb.tile([128, 512], BF16, tag="esb", bufs=2,
                      name=f"esb{half_tag}{qb}{hi}")
        nc.vector.tensor_copy(esb[:, :qw], es[:, :qw])
        nc.tensor.matmul(pr[:, :qw], lhsT=ones_bf[:], rhs=esb[:, :qw],
                         start=True, stop=True)

    rinv = tb.tile([128, 512], F32, tag="rinv", bufs=2,
                   name=f"ri{half_tag}{qb}{hi}")
    nc.vector.reciprocal_approx_fast(rinv[:, :qw], pr[:, :qw])
    nc.vector.tensor_tensor(
        out=ao_i[:, 0:qw], in0=pu[:, :qw], in1=rinv[:, :qw],
        op=mybir.AluOpType.mult)


def _out_proj_tile(nc, qb, st, agt_ab, wo_t, out, tco, psP):
    """One 128-row tile of the output projection for s-chunk qb.

    agt_ab = (agt_a, agt_b, col_off) where the agt tiles hold the
    AllToAll-delivered heads for columns [col_off, col_off+width)."""
    agt_a, agt_b, coff = agt_ab
    lo = st * 128 - coff
    po = psP.tile([128, OSL], F32, tag="pj", bufs=3, name=f"po{qb}{st}")
    for i in range(8):
        nc.tensor.matmul(
            po[:],
            lhsT=agt_a[:, i, lo:lo + 128],
            rhs=wo_t[:, i, :],
            start=(i == 0), stop=False)
    for i in range(8):
        nc.tensor.matmul(
            po[:],
            lhsT=agt_b[:, i, lo:lo + 128],
            rhs=wo_t[:, i + 8, :],
            start=False, stop=(i == 7))
    ot = tco.tile([128, OSL], F32, tag="ot", bufs=2, name=f"ot{qb}{st}")
    nc.vector.tensor_copy(ot[:], po[:])
    r0 = qb * 512 + st * 128
    nc.sync.dma_start(out=out[r0:r0 + 128, :], in_=ot[:])


def _build():
    nc = bacc.Bacc("TRN2", target_bir_lowering=False, debug=False,
                   num_devices=NCORES)

    xT0 = nc.declare_dram_parameter("xT0", [DIM, S], BF16, isOutput=False)
    xT1 = nc.declare_dram_parameter("xT1", [DIM, S], BF16, isOutput=False)
    wqT = nc.declare_dram_parameter("wqT", [DIM, HD], BF16, isOutput=False)
    wkT = nc.declare_dram_parameter("wkT", [DIM, HD], BF16, isOutput=False)
    wvT = nc.declare_dram_parameter("wvT", [DIM, HD], BF16, isOutput=False)
    woT = nc.declare_dram_parameter("woT", [DIM, OSL], BF16, isOutput=False)
    cosb = nc.declare_dram_parameter("cosb", [128, S], BF16, isOutput=False)
    sinb = nc.declare_dram_parameter("sinb", [128, S], BF16, isOutput=False)
    msk = nc.declare_dram_parameter("msk", [4, 128, 512], BF16, isOutput=False)
    out = nc.declare_dram_parameter("out", [S, OSL], F32, isOutput=True)
    xTs = (xT0, xT1)

    with tile.TileContext(nc) as tc:
        with (
            tc.tile_pool(name="res", bufs=1) as res,
            tc.tile_pool(name="xa", bufs=1) as xa,
            tc.tile_pool(name="qa", bufs=1) as qa,
            tc.tile_pool(name="ta", bufs=1) as ta,
            tc.tile_pool(name="eb", bufs=1) as eb,
            tc.tile_pool(name="tb", bufs=1) as tb,
            tc.tile_pool(name="xc", bufs=1) as xc,
            tc.tile_pool(name="tco", bufs=1) as tco,
            tc.tile_pool(name="dram", bufs=1, space="DRAM") as dram,
            tc.tile_pool(name="psP", bufs=1, space="PSUM") as psP,
        ):
            # ---- resident tiles -------------------------------------------
            msk_t = res.tile([128, 4, 512], BF16)
            ones_bf = res.tile([128, 128], BF16)
            ones_f32 = res.tile([128, 128], F32)
            warm_t = res.tile([128, 512], BF16)
            kT_t = res.tile([128, NI, S], BF16)    # rope'd K^T per instance
            v_t = res.tile([128, B * 16, HD], BF16)  # V natural [s, dv]
            wo_t = res.tile([128, NKT, OSL], BF16)
            wq_t = res.tile([128, NKT, HD], BF16)
            wk_t = res.tile([128, NKT, HD], BF16)
            wv_t = res.tile([128, NKT, HD], BF16)
            cos_t = res.tile([128, S], BF16)
            sin_t = res.tile([128, S], BF16)

            # warm-up: keep the PE HAM busy during the initial DMAs so the
            # first real matmuls run at full clock, and preload the exp
            # activation table.
            nc.vector.memset(ones_bf[:], 1.0)
            nc.vector.memset(ones_f32[:], 1.0)
            nc.scalar.memzero(warm_t[:])
            pw = psP.tile([128, 512], F32, tag="sc", bufs=3, name="pwarm")
            for w in range(20):
                nc.tensor.matmul(pw[:], lhsT=ones_bf[:], rhs=warm_t[:],
                                 start=(w == 0), stop=(w == 19))
            wexp = eb.tile([128, 512], BF16, tag="e", bufs=4, name="wexp")
            nc.scalar.activation(wexp[:], pw[:],
                                 mybir.ActivationFunctionType.Exp,
                                 scale=INV_SQRT_D)

            # input DMAs, in first-use order.  Consolidated multi-dim
            #  descriptors: each dma_start costs ~565ns of Sync-sequencer
            # issue time, so batch k-tiles per descriptor.  The startup-
            # critical loads (wq, first x chunk) are split 4 ways for
            # multi-queue parallelism.
            xr = [xTs[b].rearrange("(i p) s -> p i s", p=128)
                  for b in range(B)]
            xt_tiles = {}
            for sb in range(NSB):
                for b in range(B):
                    xt_tiles[(b, sb)] = xa.tile(
                        [128, NKT, SBW], BF16, tag="xt", bufs=2,
                        name=f"xt{b}{sb}")

            def load_xt(b, sb, ways=2):
                xt = xt_tiles[(b, sb)]
                ssl = slice(sb * SBW, (sb + 1) * SBW)
                w = NKT // ways
                for k in range(ways):
                    nc.sync.dma_start(
                        out=xt[:, k * w:(k + 1) * w, :],
                        in_=xr[b][:, k * w:(k + 1) * w, ssl])

            def load_w(dst, srcp, ways=2):
                sr = srcp.rearrange("(i p) h -> p i h", p=128)
                w = NKT // ways
                for k in range(ways):
                    nc.sync.dma_start(out=dst[:, k * w:(k + 1) * w, :],
                                      in_=sr[:, k * w:(k + 1) * w, :])

            load_w(wq_t, wqT, ways=4)
            load_xt(0, 0, ways=4)
            load_w(wk_t, wkT, ways=2)
            nc.sync.dma_start(out=cos_t[:], in_=cosb[:])
            nc.sync.dma_start(out=sin_t[:], in_=sinb[:])
            load_w(wv_t, wvT, ways=2)
            nc.sync.dma_start(out=msk_t[:],
                              in_=msk.rearrange("j p q -> p j q"))
            load_xt(1, 0, ways=2)

            def new_ag_in(qb, qw, tag):
                return dram.tile([NCORES * HD, qw], BF16, tag=f"agin{tag}",
                                 bufs=2, name=f"agin{tag}{qb}")

            def stage_store(ag_in, ao_i, hi, qw):
                """Store instance hi's attention output to its 4 AllToAll
                send slots (one broadcast descriptor).  Issued right after
                the instance's normalization so the collective's input is
                complete as soon as the last instance finishes."""
                b, j = hi // HPC, hi % HPC
                dst = ag_in.rearrange("(d r p) q -> p d r q", p=128, r=2)
                src = ao_i[:, 0:qw].rearrange("p (o q) -> p o q", o=1)
                nc.sync.dma_start(
                    out=dst[:, 4 * b:4 * b + 4, j, :],
                    in_=src.to_broadcast([128, 4, qw]))

            def stage_a2a(qb, ag_in, qw, qoff, tag):
                """Issue the AllToAll (stores already staged per-instance).
                Gather-side loads are deferred (load_agt) until the
                collective is known complete, so they never head-of-line-
                block the DMA queues."""
                ag_out = dram.tile([NCORES * HD, qw], BF16, tag=f"agout{tag}",
                                   bufs=2, name=f"agout{tag}{qb}")
                nc.gpsimd.collective_compute(
                    "AllToAll",
                    mybir.AluOpType.bypass,
                    ins=[ag_in.opt()],
                    outs=[ag_out.opt()],
                    replica_groups=GROUP,
                )
                return (ag_out, qoff, qw, tag)

            def load_agt(staged, qb, ways=1, bufs=2):
                ag_out, qoff, qw, tag = staged
                agr = ag_out.rearrange("(i p) q -> p i q", p=128)
                agt_a = xc.tile([128, 8, qw], BF16, tag=f"agta{tag}",
                                bufs=bufs, name=f"agta{tag}{qb}")
                agt_b = xc.tile([128, 8, qw], BF16, tag=f"agtb{tag}",
                                bufs=bufs, name=f"agtb{tag}{qb}")
                w = 8 // ways
                for k in range(ways):
                    nc.sync.dma_start(out=agt_a[:, k * w:(k + 1) * w, :],
                                      in_=agr[:, k * w:(k + 1) * w, :])
                for k in range(ways):
                    nc.sync.dma_start(out=agt_b[:, k * w:(k + 1) * w, :],
                                      in_=agr[:, 8 + k * w:8 + (k + 1) * w, :])
                return (agt_a, agt_b, qoff)

            # ---- pipeline over the 4 sequence chunks (ascending),
            #      out-projection trailing two chunks behind, its 128-row
            #      tiles interleaved between attention instances ------------
            staged_map = {}
            # queue of (qb, st, agt_ab) out-projection tiles ready to emit
            opq = []

            def emit_opj(n):
                for _ in range(min(n, len(opq))):
                    qb0, st0, ab = opq.pop(0)
                    _out_proj_tile(nc, qb0, st0, ab, wo_t, out, tco, psP)

            def att_pass(t, qoff, qw, half_tag, tag):
                """Attention for all 4 instances of chunk t over q-columns
                [qoff, qoff+qw), with per-instance a2a staging stores and
                one out-projection tile interleaved per instance.  Returns
                the filled a2a input tile; the caller triggers the
                collective (stage_a2a)."""
                ag_in = new_ag_in(t, qw, tag)
                for hi in range(NI):
                    ao_i = qa.tile([128, SBW], BF16, tag=f"ao{hi}", bufs=2,
                                   name=f"ao{half_tag}{t}{hi}")
                    _att_block(nc, t, hi, qoff, qw, qTc, kT_t, v_t, ao_i,
                               msk_t, ones_bf, ones_f32, eb, tb, psP,
                               half_tag=half_tag)
                    stage_store(ag_in, ao_i, hi, qw)
                    # fill exp stalls with out-projection tiles, but only
                    # after the first two instances: tiles between later
                    # instances would delay the last store, which gates the
                    # collective trigger
                    if hi < 2:
                        emit_opj(1)
                return ag_in

            for t in range(NSB):
                if t == 1:
                    load_w(wo_t, woT, ways=2)
                # gather-side loads for chunk t-2 (collective completed
                # during chunk t-1); its out-projection tiles interleave
                # with this chunk's attention
                if t >= 2:
                    agt = load_agt(staged_map[t - 2], t - 2, ways=4)
                    for st in range(4):
                        opq.append((t - 2, st, agt))
                qTc = qa.tile([128, NI, SBW], BF16, tag="qT", bufs=2,
                              name=f"qT{t}")
                for b in range(B):
                    _proj_block(nc, b, t, xt_tiles[(b, t)], wq_t, wk_t, wv_t,
                                cos_t, sin_t, qTc, kT_t, v_t, ta, psP)
                # prefetch next chunk's x (xt bufs=2: the DMA starts as soon
                # as this chunk's proj releases the previous buffer)
                if t + 1 < NSB:
                    load_xt(0, t + 1, ways=2)
                    load_xt(1, t + 1, ways=2)
                # two out-projection tiles right after proj (tensor-dense,
                # and the gather data has had a full proj phase to arrive)
                emit_opj(2)
                if t < NSB - 1:
                    ag_in = att_pass(t, 0, SBW, "", "")
                    staged_map[t] = stage_a2a(t, ag_in, SBW, 0, "")
                    emit_opj(len(opq))
                else:
                    # last chunk: asymmetric q-split (384/128).  The final
                    # collective carries only 128 columns, and only one
                    # out-projection tile depends on it; the three tiles of
                    # the wide first piece fill the wait.
                    ag_a = att_pass(t, 0, 384, "a", "ha")
                    ha = stage_a2a(t, ag_a, 384, 0, "ha")
                    agt = load_agt(staged_map[t - 1], t - 1, ways=4)
                    for st in range(4):
                        opq.append((t - 1, st, agt))
                    emit_opj(2)
                    # load the first piece's gather BEFORE the second
                    # piece's stores enter the DMA queues (it only waits on
                    # the first collective, completing mid-second-piece)
                    hat = load_agt(ha, t, ways=4, bufs=1)
                    ag_b = att_pass(t, 384, 128, "b", "hb")
                    hb = stage_a2a(t, ag_b, 128, 384, "hb")
                    opq += [(t, 0, hat), (t, 1, hat), (t, 2, hat)]
                    emit_opj(3)
                    hbt = load_agt(hb, t, ways=4, bufs=1)
                    opq.append((t, 3, hbt))
            emit_opj(len(opq))
    nc.compile()
    return nc


def _host_prep(x, wq, wk, wv, wo):
    """Build per-core input maps (host-side transposes + bf16 casts)."""
    bf = ml_dtypes.bfloat16
    # rope tables in the transposed [d, s] layout
    inv = 1.0 / (10000.0 ** (np.arange(0, D, 2, dtype=np.float64) / D))  # [64]
    ang = np.outer(np.arange(S, dtype=np.float64), inv)                  # [S, 64]
    cos = np.cos(ang).T        # [64, S]
    sin = np.sin(ang).T        # [64, S]
    cosb = np.repeat(cos, 2, axis=0).astype(np.float32)                  # [128, S]
    sinb = np.repeat(sin, 2, axis=0).astype(np.float32)
    sinb[0::2, :] *= -1.0      # even d rows: -sin ; odd rows: +sin

    ki = np.arange(128)[:, None]
    qj = np.arange(512)[None, :]
    msk_m = np.stack([(j * 128 + ki <= qj).astype(np.float32) for j in range(4)])

    xT_b = [np.ascontiguousarray(x[b].T).astype(bf) for b in range(B)]
    cosb, sinb = cosb.astype(bf), sinb.astype(bf)
    msk_m = msk_m.astype(bf)

    in_maps = []
    for c in range(NCORES):
        hrows = slice(c * HD, (c + 1) * HD)          # this core's 2 heads
        ocols = slice((c % 4) * OSL, (c % 4 + 1) * OSL)  # its output columns
        in_maps.append({
            "xT0": xT_b[0],
            "xT1": xT_b[1],
            "wqT": np.ascontiguousarray(wq[hrows].T).astype(bf),
            "wkT": np.ascontiguousarray(wk[hrows].T).astype(bf),
            "wvT": np.ascontiguousarray(wv[hrows].T).astype(bf),
            "woT": np.ascontiguousarray(wo[ocols, :].T).astype(bf),
            "cosb": cosb,
            "sinb": sinb,
            "msk": msk_m,
        })
    return in_maps


def kernel(x, wq, wk, wv, wo):
    global LAST_RESULT, _CACHED_NC
    if _CACHED_NC is None:
        _CACHED_NC = _build()
    nc = _CACHED_NC
    in_maps = _host_prep(x, wq, wk, wv, wo)
    res = run_bass_kernel_spmd(nc, in_maps, core_ids=list(range(NCORES)))
    LAST_RESULT = res
    out = np.empty((B, S, DIM), np.float32)
    for c in range(NCORES):
        bb = c // 4
        csl = slice((c % 4) * OSL, (c % 4 + 1) * OSL)
        out[bb, :, csl] = res.results[c]["out"]
    return out


# revision 28
# speedup vs baseline: 1.0252x; 1.0252x over previous
"""Distributed causal attention with RoPE for trn2 (8 NeuronCores).

Problem: B=2, S=2048, DIM=2048, H=16 heads, D=128.
  out = softmax(causal(rope(xq) @ rope(xk)^T / sqrt(D))) @ xv @ wo^T

Sharding: DP(2) x TP(4).  Cores 0-3 own batch 0, cores 4-7 batch 1; within
a batch-group each core owns 4 consecutive heads (4 local attention
instances).  Attention is fully local; the only collective is a 4-rank
AllGather per sequence chunk inside the batch group, which leaves every
core with the full attention output for its batch in head-major row order
(identical static indices on every core -> clean SPMD).  Each core then
computes one 512-column slice of the output projection for its batch.
Versus TP(8) over heads this halves the x loads, quarters the collective
staging stores (no send-side replication), and shrinks the barrier to 4
ranks.

The whole kernel is one software pipeline over the 4 sequence chunks:
  proj(chunk t) -> attention(t) -> AllGather(t) -> out-projection(t-2)
with the out-projection tiles interleaved into the attention phases so the
PE array has independent work whenever the ACT-exp stream stalls, and the
last chunk split asymmetrically (384/128) so the final collective carries
only 128 columns and gates a single out-projection tile.

Layout strategy (on-device matmuls contract over the partition axis):
  - host feeds x^T and w^T so no on-device transposes are needed
  - Q,K are produced transposed ([d, s]); RoPE pair-swap runs on the
    vector engine via stream_shuffle straight out of PSUM
  - softmax runs on transposed scores, flash-style streamed over k-tiles:
    exp on ACT, causal masking by multiply.  For small k-tile counts the
    row-sum (a partition reduction) accumulates through a bf16
    ones-matmul per k-tile; for larger counts the exp tiles are summed on
    the vector engine (f32) and a single bf16 ones-matmul per
    (instance, q-block) does the partition reduction
  - row-sum reciprocal via the fast custom-DVE Newton iteration
  - compute dtype bf16 (fp32 PSUM accumulation), output fp32
"""

import math
import sys

sys.path.insert(0, "/opt/trn_rl_repo")

import numpy as np
import ml_dtypes

import concourse.bass as bass
import concourse.mybir as mybir
import concourse.tile as tile
from concourse import bacc
from concourse.bass_utils import run_bass_kernel_spmd

BF16 = mybir.dt.bfloat16
F32 = mybir.dt.float32

B, S, DIM, H, D = 2, 2048, 2048, 16, 128
NCORES = 8
HPC = 4                  # heads per core (TP4 within the batch group)
NI = HPC                 # local attention instances = 4 (one batch)
HD = HPC * D             # local hidden slice = 512
OSL = 512                # output column slice per core
GROUPS = [[0, 1, 2, 3], [4, 5, 6, 7]]   # DP groups (one per batch)
INV_SQRT_D = 1.0 / math.sqrt(D)
SWAP_MASK = [i + 1 if i % 2 == 0 else i - 1 for i in range(32)]

SBW = 512                # s-block width (= pipeline chunk width)
NSB = S // SBW           # 4
NKT = DIM // 128         # 16 contraction tiles for projections

LAST_RESULT = None
_CACHED_NC = None


def _proj_block(nc, sb, xt, wq_t, wk_t, wv_t, cos_t, sin_t,
                qTc, kT_t, v_t, ta, psP):
    """QKV projections (+RoPE on Q,K) for s-block sb (single local batch).

    Q^T goes to the per-chunk tile qTc [128, NI, 512]; K^T to the resident
    kT_t at the chunk's s-range; V natural [s, dv=512] into v_t.
    """
    ssl = slice(sb * SBW, (sb + 1) * SBW)
    for wi, (w_t, qk) in enumerate(((wq_t, 0), (wk_t, 1))):
        for j in range(NI):
            pp = psP.tile([128, SBW], F32, tag="pj", bufs=3,
                          name=f"pp{sb}{j}{wi}")
            for i in range(NKT):
                nc.tensor.matmul(
                    pp[:],
                    lhsT=w_t[:, i, j * 128:(j + 1) * 128],
                    rhs=xt[:, i, :],
                    start=(i == 0), stop=(i == NKT - 1))
            # rope: out = x*cos + pair_swap(x)*sin_signed
            dst = qTc[:, j, :] if qk == 0 else kT_t[:, j, ssl]
            swq = ta.tile([128, SBW], F32, tag="swq", bufs=2,
                          name=f"swq{sb}{j}{wi}")
            nc.vector.stream_shuffle(swq[:], pp[:], SWAP_MASK)
            nc.vector.tensor_tensor(
                out=dst, in0=pp[:], in1=cos_t[:, ssl],
                op=mybir.AluOpType.mult)
            m2 = ta.tile([128, SBW], BF16, tag="m2", bufs=2,
                         name=f"m2_{sb}{j}{wi}")
            nc.vector.tensor_tensor(
                out=m2[:], in0=swq[:], in1=sin_t[:, ssl],
                op=mybir.AluOpType.mult)
            nc.vector.tensor_tensor(
                out=dst, in0=dst, in1=m2[:], op=mybir.AluOpType.add)
    # V in natural layout [s, dv=512]
    for m in range(4):
        pv = psP.tile([128, HD], F32, tag="pj", bufs=3, name=f"pv{sb}{m}")
        for i in range(NKT):
            nc.tensor.matmul(
                pv[:],
                lhsT=xt[:, i, m * 128:(m + 1) * 128],
                rhs=wv_t[:, i, :],
                start=(i == 0), stop=(i == NKT - 1))
        nc.scalar.copy(v_t[:, 4 * sb + m, :], pv[:])


def _att_block(nc, qb, hi, qoff, qw, qTc, kT_t, v_t, ao_i, msk_t,
               ones_bf, eb, tb, psP, half_tag=""):
    """Flash-style attention for one (q-block, head), q columns
    [qoff, qoff+qw) within the chunk.

    Streams k-tiles: scores -> exp -> mask.  The PV product is staggered
    two k-tiles behind the scores matmul to hide the ACT exp latency.
    The row-sum (partition reduction of the exp tiles) either accumulates
    through per-k-tile bf16 ones-matmuls (small nkt) or through f32 adds
    on the vector engine with a single ones-matmul at the end.
    """
    # number of k-tiles covering q < qb*512 + qoff + qw
    nkt = (qb * SBW + qoff + qw + 127) // 128
    es_route = nkt >= 8

    pr = psP.tile([128, 512], F32, tag="sc", bufs=3,
                  name=f"pr{half_tag}{qb}{hi}")
    pu = psP.tile([128, 512], F32, tag="pu", bufs=2,
                  name=f"pu{half_tag}{qb}{hi}")
    es = eb.tile([128, 512], F32, tag="es", bufs=2,
                 name=f"es{half_tag}{qb}{hi}")

    def scores(kt):
        dj = kt - 4 * qb
        # c0: first valid column (chunk-relative) for diagonal tiles,
        # clipped to this q-slice
        c0 = min(max(max(dj, 0) * 128 - qoff, 0), qw)
        pk = psP.tile([128, 512], F32, tag="sc", bufs=3,
                      name=f"pk{half_tag}{qb}{hi}{kt}")
        nc.tensor.matmul(
            pk[:, c0:qw],
            lhsT=kT_t[:, hi, kt * 128:(kt + 1) * 128],
            rhs=qTc[:, hi, qoff + c0:qoff + qw],
            start=True, stop=True)
        et = eb.tile([128, 512], BF16, tag="e", bufs=4,
                     name=f"et{half_tag}{qb}{hi}{kt}")
        nc.scalar.activation(
            et[:, c0:qw], pk[:, c0:qw], mybir.ActivationFunctionType.Exp,
            scale=INV_SQRT_D)
        if dj >= 0 and (dj + 1) * 128 > qoff + c0:
            # diagonal crosses this q-slice: zero the masked positions
            nc.vector.tensor_tensor(
                out=et[:, c0:qw], in0=et[:, c0:qw],
                in1=msk_t[:, dj, qoff + c0:qoff + qw],
                op=mybir.AluOpType.mult)
        if es_route:
            if kt == 0:
                nc.vector.tensor_copy(es[:, :qw], et[:, :qw])
            else:
                nc.vector.tensor_tensor(
                    out=es[:, c0:qw], in0=es[:, c0:qw], in1=et[:, c0:qw],
                    op=mybir.AluOpType.add)
        return et, c0

    def reduce(kt, et, c0):
        if not es_route:
            nc.tensor.matmul(pr[:, c0:qw], lhsT=ones_bf[:], rhs=et[:, c0:qw],
                             start=(kt == 0), stop=(kt == nkt - 1))
        nc.tensor.matmul(
            pu[:, c0:qw],
            lhsT=v_t[:, kt, hi * 128:(hi + 1) * 128],
            rhs=et[:, c0:qw],
            start=(kt == 0), stop=(kt == nkt - 1))

    pend = []
    for kt in range(nkt):
        pend.append((kt, scores(kt)))
        if len(pend) > 2:
            k0, (et0, c00) = pend.pop(0)
            reduce(k0, et0, c00)
    for k0, (et0, c00) in pend:
        reduce(k0, et0, c00)

    if es_route:
        # one bf16 cast + bf16 ones-matmul is ~3x cheaper on the PE than an
        # fp32 matmul (4 cycles/row); the single rounding of the final sum
        # costs <0.2% on the denominator
        esb = eb.tile([128, 512], BF16, tag="esb", bufs=1,
                      name=f"esb{half_tag}{qb}{hi}")
        nc.vector.tensor_copy(esb[:, :qw], es[:, :qw])
        nc.tensor.matmul(pr[:, :qw], lhsT=ones_bf[:], rhs=esb[:, :qw],
                         start=True, stop=True)

    rinv = tb.tile([128, 512], F32, tag="rinv", bufs=1,
                   name=f"ri{half_tag}{qb}{hi}")
    nc.vector.reciprocal_approx_fast(rinv[:, :qw], pr[:, :qw])
    nc.vector.tensor_tensor(
        out=ao_i[:, 0:qw], in0=pu[:, :qw], in1=rinv[:, :qw],
        op=mybir.AluOpType.mult)


def _out_proj_tile(nc, qb, st, agt_ab, wo_t, out, tco, psP):
    """One 128-row tile of the output projection for s-chunk qb.

    agt_ab = (agt_a, agt_b, col_off) where the agt tiles hold the
    AllGather-delivered heads for columns [col_off, col_off+width)."""
    agt_a, agt_b, coff = agt_ab
    lo = st * 128 - coff
    po = psP.tile([128, OSL], F32, tag="pj", bufs=3, name=f"po{qb}{st}")
    for i in range(8):
        nc.tensor.matmul(
            po[:],
            lhsT=agt_a[:, i, lo:lo + 128],
            rhs=wo_t[:, i, :],
            start=(i == 0), stop=False)
    for i in range(8):
        nc.tensor.matmul(
            po[:],
            lhsT=agt_b[:, i, lo:lo + 128],
            rhs=wo_t[:, i + 8, :],
            start=False, stop=(i == 7))
    ot = tco.tile([128, OSL], F32, tag="ot", bufs=1, name=f"ot{qb}{st}")
    nc.vector.tensor_copy(ot[:], po[:])
    r0 = qb * 512 + st * 128
    nc.sync.dma_start(out=out[r0:r0 + 128, :], in_=ot[:])


def _build():
    nc = bacc.Bacc("TRN2", target_bir_lowering=False, debug=False,
                   num_devices=NCORES)

    xT = nc.declare_dram_parameter("xT", [DIM, S], BF16, isOutput=False)
    wqT = nc.declare_dram_parameter("wqT", [DIM, HD], BF16, isOutput=False)
    wkT = nc.declare_dram_parameter("wkT", [DIM, HD], BF16, isOutput=False)
    wvT = nc.declare_dram_parameter("wvT", [DIM, HD], BF16, isOutput=False)
    woT = nc.declare_dram_parameter("woT", [DIM, OSL], BF16, isOutput=False)
    cosb = nc.declare_dram_parameter("cosb", [128, S], BF16, isOutput=False)
    sinb = nc.declare_dram_parameter("sinb", [128, S], BF16, isOutput=False)
    msk = nc.declare_dram_parameter("msk", [4, 128, 512], BF16, isOutput=False)
    out = nc.declare_dram_parameter("out", [S, OSL], F32, isOutput=True)

    with tile.TileContext(nc) as tc:
        with (
            tc.tile_pool(name="res", bufs=1) as res,
            tc.tile_pool(name="xa", bufs=1) as xa,
            tc.tile_pool(name="qa", bufs=1) as qa,
            tc.tile_pool(name="ta", bufs=1) as ta,
            tc.tile_pool(name="eb", bufs=1) as eb,
            tc.tile_pool(name="tb", bufs=1) as tb,
            tc.tile_pool(name="xc", bufs=1) as xc,
            tc.tile_pool(name="tco", bufs=1) as tco,
            tc.tile_pool(name="dram", bufs=1, space="DRAM") as dram,
            tc.tile_pool(name="psP", bufs=1, space="PSUM") as psP,
        ):
            # ---- resident tiles -------------------------------------------
            msk_t = res.tile([128, 4, 512], BF16)
            ones_bf = res.tile([128, 128], BF16)
            kT_t = res.tile([128, NI, S], BF16)    # rope'd K^T per head
            v_t = res.tile([128, 16, HD], BF16)    # V natural [s, dv=512]
            wo_t = res.tile([128, NKT, OSL], BF16)
            wq_t = res.tile([128, NKT, HD], BF16)
            wk_t = res.tile([128, NKT, HD], BF16)
            wv_t = res.tile([128, NKT, HD], BF16)
            cos_t = res.tile([128, S], BF16)
            sin_t = res.tile([128, S], BF16)

            # warm-up: keep the PE HAM busy during the initial DMAs so the
            # first real matmuls run at full clock, and preload the exp
            # activation table.
            nc.vector.memset(ones_bf[:], 1.0)
            pw = psP.tile([128, 512], F32, tag="sc", bufs=3, name="pwarm")
            for w in range(60):
                nc.tensor.matmul(pw[:, :128], lhsT=ones_bf[:],
                                 rhs=ones_bf[:],
                                 start=(w == 0), stop=(w == 59))
            wexp = eb.tile([128, 512], BF16, tag="e", bufs=4, name="wexp")
            nc.scalar.activation(wexp[:, :128], pw[:, :128],
                                 mybir.ActivationFunctionType.Exp,
                                 scale=INV_SQRT_D)

            # input DMAs, in first-use order.  Consolidated multi-dim
            # descriptors: each dma_start costs ~565ns of Sync-sequencer
            # issue time, so batch k-tiles per descriptor.  The startup-
            # critical loads (wq, first x chunk) are split 4 ways for
            # multi-queue parallelism.
            xr = xT.rearrange("(i p) s -> p i s", p=128)
            xt_tiles = {}
            for sb in range(NSB):
                xt_tiles[sb] = xa.tile([128, NKT, SBW], BF16, tag="xt",
                                       bufs=2, name=f"xt{sb}")

            def load_xt(sb, ways=2):
                xt = xt_tiles[sb]
                ssl = slice(sb * SBW, (sb + 1) * SBW)
                w = NKT // ways
                for k in range(ways):
                    nc.sync.dma_start(
                        out=xt[:, k * w:(k + 1) * w, :],
                        in_=xr[:, k * w:(k + 1) * w, ssl])

            def load_w(dst, srcp, ways=2):
                sr = srcp.rearrange("(i p) h -> p i h", p=128)
                w = NKT // ways
                for k in range(ways):
                    nc.sync.dma_start(out=dst[:, k * w:(k + 1) * w, :],
                                      in_=sr[:, k * w:(k + 1) * w, :])

            load_w(wq_t, wqT, ways=4)
            load_xt(0, ways=4)
            load_w(wk_t, wkT, ways=2)
            nc.sync.dma_start(out=cos_t[:], in_=cosb[:])
            nc.sync.dma_start(out=sin_t[:], in_=sinb[:])
            load_w(wv_t, wvT, ways=2)
            nc.sync.dma_start(out=msk_t[:],
                              in_=msk.rearrange("j p q -> p j q"))

            def new_ag_in(qb, qw, tag):
                return dram.tile([HD, qw], BF16, tag=f"agin{tag}",
                                 bufs=2, name=f"agin{tag}{qb}")

            def stage_store(ag_in, ao_i, hi, qw):
                """Store head hi's attention output to its AllGather send
                slot.  Issued right after the instance's normalization so
                the collective's input is complete as soon as the last
                instance finishes."""
                nc.sync.dma_start(
                    out=ag_in[hi * 128:(hi + 1) * 128, :],
                    in_=ao_i[:, 0:qw])

            def stage_ag(qb, ag_in, qw, qoff, tag):
                """Issue the 4-rank AllGather (stores already staged
                per-instance).  Gather-side loads are deferred (load_agt)
                until the collective is known complete, so they never
                head-of-line-block the DMA queues."""
                ag_out = dram.tile([4 * HD, qw], BF16, tag=f"agout{tag}",
                                   bufs=2, name=f"agout{tag}{qb}")
                nc.gpsimd.collective_compute(
                    "AllGather",
                    mybir.AluOpType.bypass,
                    ins=[ag_in.opt()],
                    outs=[ag_out.opt()],
                    replica_groups=GROUPS,
                )
                return (ag_out, qoff, qw, tag)

            def load_agt(staged, qb, ways=1, bufs=2):
                ag_out, qoff, qw, tag = staged
                agr = ag_out.rearrange("(i p) q -> p i q", p=128)
                agt_a = xc.tile([128, 8, qw], BF16, tag=f"agta{tag}",
                                bufs=bufs, name=f"agta{tag}{qb}")
                agt_b = xc.tile([128, 8, qw], BF16, tag=f"agtb{tag}",
                                bufs=bufs, name=f"agtb{tag}{qb}")
                w = 8 // ways
                for k in range(ways):
                    nc.sync.dma_start(out=agt_a[:, k * w:(k + 1) * w, :],
                                      in_=agr[:, k * w:(k + 1) * w, :])
                for k in range(ways):
                    nc.sync.dma_start(out=agt_b[:, k * w:(k + 1) * w, :],
                                      in_=agr[:, 8 + k * w:8 + (k + 1) * w, :])
                return (agt_a, agt_b, qoff)

            # ---- pipeline over the 4 sequence chunks (ascending),
            #      out-projection trailing two chunks behind, its 128-row
            #      tiles interleaved into the attention phases ---------------
            staged_map = {}
            # queue of (qb, st, agt_ab) out-projection tiles ready to emit
            opq = []

            def emit_opj(n):
                for _ in range(min(n, len(opq))):
                    qb0, st0, ab = opq.pop(0)
                    _out_proj_tile(nc, qb0, st0, ab, wo_t, out, tco, psP)

            def att_pass(t, qoff, qw, half_tag, tag):
                """Attention for all 4 heads of chunk t over q-columns
                [qoff, qoff+qw), with per-instance staging stores and out-
                projection tiles interleaved after the first two heads."""
                ag_in = new_ag_in(t, qw, tag)
                for hi in range(NI):
                    ao_i = qa.tile([128, SBW], BF16, tag=f"ao{hi}", bufs=2,
                                   name=f"ao{half_tag}{t}{hi}")
                    _att_block(nc, t, hi, qoff, qw, qTc, kT_t, v_t, ao_i,
                               msk_t, ones_bf, eb, tb, psP,
                               half_tag=half_tag)
                    stage_store(ag_in, ao_i, hi, qw)
                    # fill exp stalls with out-projection tiles, but only
                    # after the first two heads: tiles between later heads
                    # would delay the last store, which gates the
                    # collective trigger
                    if hi < 2:
                        emit_opj(1)
                return ag_in

            for t in range(NSB):
                if t == 1:
                    load_w(wo_t, woT, ways=2)
                # gather-side loads for chunk t-2 (collective completed
                # during chunk t-1); its out-projection tiles interleave
                # with this chunk's attention
                if t >= 2:
                    agt = load_agt(staged_map[t - 2], t - 2, ways=4, bufs=1)
                    for st in range(4):
                        opq.append((t - 2, st, agt))
                qTc = qa.tile([128, NI, SBW], BF16, tag="qT", bufs=2,
                              name=f"qT{t}")
                _proj_block(nc, t, xt_tiles[t], wq_t, wk_t, wv_t,
                            cos_t, sin_t, qTc, kT_t, v_t, ta, psP)
                # prefetch next chunk's x (xt bufs=2: the DMA starts as soon
                # as this chunk's proj releases the previous buffer)
                if t + 1 < NSB:
                    load_xt(t + 1, ways=2)
                # two out-projection tiles right after proj (tensor-dense,
                # and the gather data has had a full proj phase to arrive)
                emit_opj(2)
                if t < NSB - 1:
                    ag_in = att_pass(t, 0, SBW, "", "")
                    staged_map[t] = stage_ag(t, ag_in, SBW, 0, "")
                    emit_opj(len(opq))
                else:
                    # last chunk: asymmetric q-split (384/128).  The final
                    # collective carries only 128 columns, and only one
                    # out-projection tile depends on it; the three tiles of
                    # the wide first piece fill the wait.
                    ag_a = att_pass(t, 0, 384, "a", "ha")
                    ha = stage_ag(t, ag_a, 384, 0, "ha")
                    agt = load_agt(staged_map[t - 1], t - 1, ways=4, bufs=1)
                    for st in range(4):
                        opq.append((t - 1, st, agt))
                    emit_opj(2)
                    # load the first piece's gather BEFORE the second
                    # piece's stores enter the DMA queues (it only waits on
                    # the first collective, completing mid-second-piece)
                    hat = load_agt(ha, t, ways=4, bufs=1)
                    ag_b = att_pass(t, 384, 128, "b", "hb")
                    hb = stage_ag(t, ag_b, 128, 384, "hb")
                    opq += [(t, 0, hat), (t, 1, hat), (t, 2, hat)]
                    emit_opj(3)
                    hbt = load_agt(hb, t, ways=4, bufs=1)
                    opq.append((t, 3, hbt))
            emit_opj(len(opq))
    nc.compile()
    return nc


def _host_prep(x, wq, wk, wv, wo):
    """Build per-core input maps (host-side transposes + bf16 casts)."""
    bf = ml_dtypes.bfloat16
    # rope tables in the transposed [d, s] layout
    inv = 1.0 / (10000.0 ** (np.arange(0, D, 2, dtype=np.float64) / D))  # [64]
    ang = np.outer(np.arange(S, dtype=np.float64), inv)                  # [S, 64]
    cos = np.cos(ang).T        # [64, S]
    sin = np.sin(ang).T        # [64, S]
    cosb = np.repeat(cos, 2, axis=0).astype(np.float32)                  # [128, S]
    sinb = np.repeat(sin, 2, axis=0).astype(np.float32)
    sinb[0::2, :] *= -1.0      # even d rows: -sin ; odd rows: +sin

    ki = np.arange(128)[:, None]
    qj = np.arange(512)[None, :]
    msk_m = np.stack([(j * 128 + ki <= qj).astype(np.float32) for j in range(4)])

    xT_b = [np.ascontiguousarray(x[b].T).astype(bf) for b in range(B)]
    cosb, sinb = cosb.astype(bf), sinb.astype(bf)
    msk_m = msk_m.astype(bf)

    in_maps = []
    for c in range(NCORES):
        hrows = slice((c % 4) * HD, (c % 4 + 1) * HD)     # this core's 4 heads
        ocols = slice((c % 4) * OSL, (c % 4 + 1) * OSL)   # its output columns
        in_maps.append({
            "xT": xT_b[c // 4],
            "wqT": np.ascontiguousarray(wq[hrows].T).astype(bf),
            "wkT": np.ascontiguousarray(wk[hrows].T).astype(bf),
            "wvT": np.ascontiguousarray(wv[hrows].T).astype(bf),
            "woT": np.ascontiguousarray(wo[ocols, :].T).astype(bf),
            "cosb": cosb,
            "sinb": sinb,
            "msk": msk_m,
        })
    return in_maps


def kernel(x, wq, wk, wv, wo):
    global LAST_RESULT, _CACHED_NC
    if _CACHED_NC is None:
        _CACHED_NC = _build()
    nc = _CACHED_NC
    in_maps = _host_prep(x, wq, wk, wv, wo)
    res = run_bass_kernel_spmd(nc, in_maps, core_ids=list(range(NCORES)))
    LAST_RESULT = res
    out = np.empty((B, S, DIM), np.float32)
    for c in range(NCORES):
        bb = c // 4
        csl = slice((c % 4) * OSL, (c % 4 + 1) * OSL)
        out[bb, :, csl] = res.results[c]["out"]
    return out


# revision 30
# speedup vs baseline: 1.0486x; 1.0229x over previous
"""Distributed causal attention with RoPE for trn2 (8 NeuronCores).

Problem: B=2, S=2048, DIM=2048, H=16 heads, D=128.
  out = softmax(causal(rope(xq) @ rope(xk)^T / sqrt(D))) @ xv @ wo^T

Sharding: DP(2) x TP(4).  Cores 0-3 own batch 0, cores 4-7 batch 1; within
a batch-group each core owns 4 consecutive heads (4 local attention
instances).  Attention is fully local; the only collective is a 4-rank
AllGather per sequence chunk inside the batch group, which leaves every
core with the full attention output for its batch in head-major row order
(identical static indices on every core -> clean SPMD).  Each core then
computes one 512-column slice of the output projection for its batch.
Versus TP(8) over heads this halves the x loads, quarters the collective
staging stores (no send-side replication), and shrinks the barrier to 4
ranks.

The whole kernel is one software pipeline over the 4 sequence chunks:
  proj(chunk t) -> attention(t) -> AllGather(t) -> out-projection(t-2)
with the out-projection tiles interleaved into the attention phases so the
PE array has independent work whenever the ACT-exp stream stalls, and the
last chunk split asymmetrically (384/128) so the final collective carries
only 128 columns and gates a single out-projection tile.

Layout strategy (on-device matmuls contract over the partition axis):
  - host feeds x^T and w^T so no on-device transposes are needed
  - Q,K are produced transposed ([d, s]); RoPE pair-swap runs on the
    vector engine via stream_shuffle straight out of PSUM
  - softmax runs on transposed scores, flash-style streamed over k-tiles:
    exp on ACT, causal masking by multiply.  For small k-tile counts the
    row-sum (a partition reduction) accumulates through a bf16
    ones-matmul per k-tile; for larger counts the exp tiles are summed on
    the vector engine (f32) and a single bf16 ones-matmul per
    (instance, q-block) does the partition reduction
  - row-sum reciprocal via the fast custom-DVE Newton iteration
  - compute dtype bf16 (fp32 PSUM accumulation), output fp32
"""

import math
import sys

sys.path.insert(0, "/opt/trn_rl_repo")

import numpy as np
import ml_dtypes

import concourse.bass as bass
import concourse.mybir as mybir
import concourse.tile as tile
from concourse import bacc
from concourse.bass_utils import run_bass_kernel_spmd

BF16 = mybir.dt.bfloat16
F32 = mybir.dt.float32

B, S, DIM, H, D = 2, 2048, 2048, 16, 128
NCORES = 8
HPC = 4                  # heads per core (TP4 within the batch group)
NI = HPC                 # local attention instances = 4 (one batch)
HD = HPC * D             # local hidden slice = 512
OSL = 512                # output column slice per core
GROUPS = [[0, 1, 2, 3], [4, 5, 6, 7]]   # DP groups (one per batch)
INV_SQRT_D = 1.0 / math.sqrt(D)
SWAP_MASK = [i + 1 if i % 2 == 0 else i - 1 for i in range(32)]

SBW = 512                # s-block width (= pipeline chunk width)
NSB = S // SBW           # 4
NKT = DIM // 128         # 16 contraction tiles for projections

LAST_RESULT = None
_CACHED_NC = None


def _proj_block(nc, sb, xt, wq_t, wk_t, wv_t, cos_t, sin_t,
                qTc, kT_t, v_t, ta, psP):
    """QKV projections (+RoPE on Q,K) for s-block sb (single local batch).

    Q^T goes to the per-chunk tile qTc [128, NI, 512]; K^T to the resident
    kT_t at the chunk's s-range; V natural [s, dv=512] into v_t.
    """
    ssl = slice(sb * SBW, (sb + 1) * SBW)
    for wi, (w_t, qk) in enumerate(((wq_t, 0), (wk_t, 1))):
        for j in range(NI):
            pp = psP.tile([128, SBW], F32, tag="pj", bufs=3,
                          name=f"pp{sb}{j}{wi}")
            for i in range(NKT):
                nc.tensor.matmul(
                    pp[:],
                    lhsT=w_t[:, i, j * 128:(j + 1) * 128],
                    rhs=xt[:, i, :],
                    start=(i == 0), stop=(i == NKT - 1))
            # rope: out = x*cos + pair_swap(x)*sin_signed
            dst = qTc[:, j, :] if qk == 0 else kT_t[:, j, ssl]
            swq = ta.tile([128, SBW], F32, tag="swq", bufs=2,
                          name=f"swq{sb}{j}{wi}")
            nc.vector.stream_shuffle(swq[:], pp[:], SWAP_MASK)
            nc.vector.tensor_tensor(
                out=dst, in0=pp[:], in1=cos_t[:, ssl],
                op=mybir.AluOpType.mult)
            m2 = ta.tile([128, SBW], BF16, tag="m2", bufs=2,
                         name=f"m2_{sb}{j}{wi}")
            nc.vector.tensor_tensor(
                out=m2[:], in0=swq[:], in1=sin_t[:, ssl],
                op=mybir.AluOpType.mult)
            nc.vector.tensor_tensor(
                out=dst, in0=dst, in1=m2[:], op=mybir.AluOpType.add)
    # V in natural layout [s, dv=512]
    for m in range(4):
        pv = psP.tile([128, HD], F32, tag="pj", bufs=3, name=f"pv{sb}{m}")
        for i in range(NKT):
            nc.tensor.matmul(
                pv[:],
                lhsT=xt[:, i, m * 128:(m + 1) * 128],
                rhs=wv_t[:, i, :],
                start=(i == 0), stop=(i == NKT - 1))
        nc.scalar.copy(v_t[:, 4 * sb + m, :], pv[:])


def _att_block(nc, qb, hi, qoff, qw, qTc, kT_t, v_t, ao_i, msk_t,
               ones_bf, eb, tb, psP, half_tag=""):
    """Flash-style attention for one (q-block, head), q columns
    [qoff, qoff+qw) within the chunk.

    Streams k-tiles: scores -> exp -> mask.  The PV product is staggered
    two k-tiles behind the scores matmul to hide the ACT exp latency.
    The row-sum (partition reduction of the exp tiles) either accumulates
    through per-k-tile bf16 ones-matmuls (small nkt) or through f32 adds
    on the vector engine with a single ones-matmul at the end.
    """
    # number of k-tiles covering q < qb*512 + qoff + qw
    nkt = (qb * SBW + qoff + qw + 127) // 128
    es_route = nkt >= 8

    pr = psP.tile([128, 512], F32, tag="sc", bufs=3,
                  name=f"pr{half_tag}{qb}{hi}")
    pu = psP.tile([128, 512], F32, tag="pu", bufs=2,
                  name=f"pu{half_tag}{qb}{hi}")
    es = eb.tile([128, 512], F32, tag="es", bufs=2,
                 name=f"es{half_tag}{qb}{hi}")

    def scores(kt):
        dj = kt - 4 * qb
        # c0: first valid column (chunk-relative) for diagonal tiles,
        # clipped to this q-slice
        c0 = min(max(max(dj, 0) * 128 - qoff, 0), qw)
        pk = psP.tile([128, 512], F32, tag="sc", bufs=3,
                      name=f"pk{half_tag}{qb}{hi}{kt}")
        nc.tensor.matmul(
            pk[:, c0:qw],
            lhsT=kT_t[:, hi, kt * 128:(kt + 1) * 128],
            rhs=qTc[:, hi, qoff + c0:qoff + qw],
            start=True, stop=True)
        et = eb.tile([128, 512], BF16, tag="e", bufs=4,
                     name=f"et{half_tag}{qb}{hi}{kt}")
        nc.scalar.activation(
            et[:, c0:qw], pk[:, c0:qw], mybir.ActivationFunctionType.Exp,
            scale=INV_SQRT_D)
        if dj >= 0 and (dj + 1) * 128 > qoff + c0:
            # diagonal crosses this q-slice: zero the masked positions
            nc.vector.tensor_tensor(
                out=et[:, c0:qw], in0=et[:, c0:qw],
                in1=msk_t[:, dj, qoff + c0:qoff + qw],
                op=mybir.AluOpType.mult)
        if es_route:
            if kt == 0:
                nc.vector.tensor_copy(es[:, :qw], et[:, :qw])
            else:
                nc.vector.tensor_tensor(
                    out=es[:, c0:qw], in0=es[:, c0:qw], in1=et[:, c0:qw],
                    op=mybir.AluOpType.add)
        return et, c0

    def reduce(kt, et, c0):
        if not es_route:
            nc.tensor.matmul(pr[:, c0:qw], lhsT=ones_bf[:], rhs=et[:, c0:qw],
                             start=(kt == 0), stop=(kt == nkt - 1))
        nc.tensor.matmul(
            pu[:, c0:qw],
            lhsT=v_t[:, kt, hi * 128:(hi + 1) * 128],
            rhs=et[:, c0:qw],
            start=(kt == 0), stop=(kt == nkt - 1))

    pend = []
    for kt in range(nkt):
        pend.append((kt, scores(kt)))
        if len(pend) > 2:
            k0, (et0, c00) = pend.pop(0)
            reduce(k0, et0, c00)
    for k0, (et0, c00) in pend:
        reduce(k0, et0, c00)

    if es_route:
        # one bf16 cast + bf16 ones-matmul is ~3x cheaper on the PE than an
        # fp32 matmul (4 cycles/row); the single rounding of the final sum
        # costs <0.2% on the denominator
        esb = eb.tile([128, 512], BF16, tag="esb", bufs=1,
                      name=f"esb{half_tag}{qb}{hi}")
        nc.vector.tensor_copy(esb[:, :qw], es[:, :qw])
        nc.tensor.matmul(pr[:, :qw], lhsT=ones_bf[:], rhs=esb[:, :qw],
                         start=True, stop=True)

    rinv = tb.tile([128, 512], F32, tag="rinv", bufs=1,
                   name=f"ri{half_tag}{qb}{hi}")
    nc.vector.reciprocal_approx_fast(rinv[:, :qw], pr[:, :qw])
    nc.vector.tensor_tensor(
        out=ao_i[:, 0:qw], in0=pu[:, :qw], in1=rinv[:, :qw],
        op=mybir.AluOpType.mult)


def _out_proj_tile(nc, qb, st, agt_ab, wo_t, out, tco, psP):
    """One 128-row tile of the output projection for s-chunk qb.

    agt_ab = (agt_a, agt_b, col_off) where the agt tiles hold the
    AllGather-delivered heads for columns [col_off, col_off+width)."""
    agt_a, agt_b, coff = agt_ab
    lo = st * 128 - coff
    po = psP.tile([128, OSL], F32, tag="pj", bufs=3, name=f"po{qb}{st}")
    for i in range(8):
        nc.tensor.matmul(
            po[:],
            lhsT=agt_a[:, i, lo:lo + 128],
            rhs=wo_t[:, i, :],
            start=(i == 0), stop=False)
    for i in range(8):
        nc.tensor.matmul(
            po[:],
            lhsT=agt_b[:, i, lo:lo + 128],
            rhs=wo_t[:, i + 8, :],
            start=False, stop=(i == 7))
    ot = tco.tile([128, OSL], F32, tag="ot", bufs=1, name=f"ot{qb}{st}")
    nc.vector.tensor_copy(ot[:], po[:])
    r0 = qb * 512 + st * 128
    nc.sync.dma_start(out=out[r0:r0 + 128, :], in_=ot[:])


def _build():
    nc = bacc.Bacc("TRN2", target_bir_lowering=False, debug=False,
                   num_devices=NCORES)

    xT = nc.declare_dram_parameter("xT", [DIM, S], BF16, isOutput=False)
    wqT = nc.declare_dram_parameter("wqT", [DIM, HD], BF16, isOutput=False)
    wkT = nc.declare_dram_parameter("wkT", [DIM, HD], BF16, isOutput=False)
    wvT = nc.declare_dram_parameter("wvT", [DIM, HD], BF16, isOutput=False)
    woT = nc.declare_dram_parameter("woT", [DIM, OSL], BF16, isOutput=False)
    cosb = nc.declare_dram_parameter("cosb", [128, S], BF16, isOutput=False)
    sinb = nc.declare_dram_parameter("sinb", [128, S], BF16, isOutput=False)
    msk = nc.declare_dram_parameter("msk", [4, 128, 512], BF16, isOutput=False)
    out = nc.declare_dram_parameter("out", [S, OSL], F32, isOutput=True)

    with tile.TileContext(nc) as tc:
        with (
            tc.tile_pool(name="res", bufs=1) as res,
            tc.tile_pool(name="xa", bufs=1) as xa,
            tc.tile_pool(name="qa", bufs=1) as qa,
            tc.tile_pool(name="ta", bufs=1) as ta,
            tc.tile_pool(name="eb", bufs=1) as eb,
            tc.tile_pool(name="tb", bufs=1) as tb,
            tc.tile_pool(name="xc", bufs=1) as xc,
            tc.tile_pool(name="tco", bufs=1) as tco,
            tc.tile_pool(name="dram", bufs=1, space="DRAM") as dram,
            tc.tile_pool(name="psP", bufs=1, space="PSUM") as psP,
        ):
            # ---- resident tiles -------------------------------------------
            msk_t = res.tile([128, 4, 512], BF16)
            ones_bf = res.tile([128, 128], BF16)
            kT_t = res.tile([128, NI, S], BF16)    # rope'd K^T per head
            v_t = res.tile([128, 16, HD], BF16)    # V natural [s, dv=512]
            wo_t = res.tile([128, NKT, OSL], BF16)
            wq_t = res.tile([128, NKT, HD], BF16)
            wk_t = res.tile([128, NKT, HD], BF16)
            wv_t = res.tile([128, NKT, HD], BF16)
            cos_t = res.tile([128, S], BF16)
            sin_t = res.tile([128, S], BF16)

            # warm-up: keep the PE HAM busy during the initial DMAs so the
            # first real matmuls run at full clock, and preload the exp
            # activation table.
            nc.vector.memset(ones_bf[:], 1.0)
            pw = psP.tile([128, 512], F32, tag="sc", bufs=3, name="pwarm")
            for w in range(60):
                nc.tensor.matmul(pw[:, :128], lhsT=ones_bf[:],
                                 rhs=ones_bf[:],
                                 start=(w == 0), stop=(w == 59))
            wexp = eb.tile([128, 512], BF16, tag="e", bufs=4, name="wexp")
            nc.scalar.activation(wexp[:, :128], pw[:, :128],
                                 mybir.ActivationFunctionType.Exp,
                                 scale=INV_SQRT_D)

            # input DMAs, in first-use order.  Consolidated multi-dim
            # descriptors: each dma_start costs ~565ns of Sync-sequencer
            # issue time, so batch k-tiles per descriptor.  The startup-
            # critical loads (wq, first x chunk) are split 4 ways for
            # multi-queue parallelism.
            xr = xT.rearrange("(i p) s -> p i s", p=128)
            xt_tiles = {}
            for sb in range(NSB):
                xt_tiles[sb] = xa.tile([128, NKT, SBW], BF16, tag="xt",
                                       bufs=2, name=f"xt{sb}")

            def load_xt(sb, ways=2):
                xt = xt_tiles[sb]
                ssl = slice(sb * SBW, (sb + 1) * SBW)
                w = NKT // ways
                for k in range(ways):
                    nc.sync.dma_start(
                        out=xt[:, k * w:(k + 1) * w, :],
                        in_=xr[:, k * w:(k + 1) * w, ssl])

            def load_w(dst, srcp, ways=2):
                sr = srcp.rearrange("(i p) h -> p i h", p=128)
                w = NKT // ways
                for k in range(ways):
                    nc.sync.dma_start(out=dst[:, k * w:(k + 1) * w, :],
                                      in_=sr[:, k * w:(k + 1) * w, :])

            load_w(wq_t, wqT, ways=4)
            load_xt(0, ways=4)
            load_w(wk_t, wkT, ways=2)
            nc.sync.dma_start(out=cos_t[:], in_=cosb[:])
            nc.sync.dma_start(out=sin_t[:], in_=sinb[:])
            load_w(wv_t, wvT, ways=2)
            nc.sync.dma_start(out=msk_t[:],
                              in_=msk.rearrange("j p q -> p j q"))

            def new_ag_in(qb, qw, tag):
                return dram.tile([HD, qw], BF16, tag=f"agin{tag}",
                                 bufs=2, name=f"agin{tag}{qb}")

            def stage_store(ag_in, ao_i, hi, qw):
                """Store head hi's attention output to its AllGather send
                slot.  Issued right after the instance's normalization so
                the collective's input is complete as soon as the last
                instance finishes."""
                nc.sync.dma_start(
                    out=ag_in[hi * 128:(hi + 1) * 128, :],
                    in_=ao_i[:, 0:qw])

            def stage_ag(qb, ag_in, qw, qoff, tag):
                """Issue the 4-rank AllGather (stores already staged
                per-instance).  Gather-side loads are deferred (load_agt)
                until the collective is known complete, so they never
                head-of-line-block the DMA queues."""
                ag_out = dram.tile([4 * HD, qw], BF16, tag=f"agout{tag}",
                                   bufs=2, name=f"agout{tag}{qb}")
                nc.gpsimd.collective_compute(
                    "AllGather",
                    mybir.AluOpType.bypass,
                    ins=[ag_in.opt()],
                    outs=[ag_out.opt()],
                    replica_groups=GROUPS,
                )
                return (ag_out, qoff, qw, tag)

            def load_agt(staged, qb, ways=1, bufs=2):
                ag_out, qoff, qw, tag = staged
                agr = ag_out.rearrange("(i p) q -> p i q", p=128)
                agt_a = xc.tile([128, 8, qw], BF16, tag=f"agta{tag}",
                                bufs=bufs, name=f"agta{tag}{qb}")
                agt_b = xc.tile([128, 8, qw], BF16, tag=f"agtb{tag}",
                                bufs=bufs, name=f"agtb{tag}{qb}")
                w = 8 // ways
                for k in range(ways):
                    nc.sync.dma_start(out=agt_a[:, k * w:(k + 1) * w, :],
                                      in_=agr[:, k * w:(k + 1) * w, :])
                for k in range(ways):
                    nc.sync.dma_start(out=agt_b[:, k * w:(k + 1) * w, :],
                                      in_=agr[:, 8 + k * w:8 + (k + 1) * w, :])
                return (agt_a, agt_b, qoff)

            # ---- pipeline over the 4 sequence chunks (ascending),
            #      out-projection trailing two chunks behind, its 128-row
            #      tiles interleaved into the attention phases ---------------
            staged_map = {}
            # queue of (qb, st, agt_ab) out-projection tiles ready to emit
            opq = []

            def emit_opj(n):
                for _ in range(min(n, len(opq))):
                    qb0, st0, ab = opq.pop(0)
                    _out_proj_tile(nc, qb0, st0, ab, wo_t, out, tco, psP)

            def att_pass(t, qoff, qw, half_tag, tag, fill=True):
                """Attention for all 4 heads of chunk t over q-columns
                [qoff, qoff+qw), with per-instance staging stores and
                (optionally) out-projection tiles interleaved after the
                first two heads.  The tail pieces pass fill=False: any tile
                before the last head delays the last store, which gates
                the collective trigger that the tail waits on."""
                ag_in = new_ag_in(t, qw, tag)
                for hi in range(NI):
                    ao_i = qa.tile([128, SBW], BF16, tag=f"ao{hi}", bufs=2,
                                   name=f"ao{half_tag}{t}{hi}")
                    _att_block(nc, t, hi, qoff, qw, qTc, kT_t, v_t, ao_i,
                               msk_t, ones_bf, eb, tb, psP,
                               half_tag=half_tag)
                    stage_store(ag_in, ao_i, hi, qw)
                    if fill and hi < 2:
                        emit_opj(1)
                return ag_in

            for t in range(NSB):
                if t == 1:
                    load_w(wo_t, woT, ways=2)
                # gather-side loads for chunk t-2 (collective completed
                # during chunk t-1); its out-projection tiles interleave
                # with this chunk's attention
                if t >= 2:
                    agt = load_agt(staged_map[t - 2], t - 2, ways=4, bufs=1)
                    for st in range(4):
                        opq.append((t - 2, st, agt))
                qTc = qa.tile([128, NI, SBW], BF16, tag="qT", bufs=2,
                              name=f"qT{t}")
                _proj_block(nc, t, xt_tiles[t], wq_t, wk_t, wv_t,
                            cos_t, sin_t, qTc, kT_t, v_t, ta, psP)
                # prefetch next chunk's x (xt bufs=2: the DMA starts as soon
                # as this chunk's proj releases the previous buffer)
                if t + 1 < NSB:
                    load_xt(t + 1, ways=2)
                # two out-projection tiles right after proj (tensor-dense,
                # and the gather data has had a full proj phase to arrive)
                emit_opj(2)
                if t < NSB - 1:
                    ag_in = att_pass(t, 0, SBW, "", "")
                    staged_map[t] = stage_ag(t, ag_in, SBW, 0, "")
                    emit_opj(len(opq))
                else:
                    # last chunk: asymmetric q-split (384/128).  The final
                    # collective carries only 128 columns, and only one
                    # out-projection tile depends on it.  Both pieces run
                    # with fill=False so their collectives trigger as early
                    # as possible; the queued tiles then fill the
                    # collectives' data phases.
                    ag_a = att_pass(t, 0, 384, "a", "ha", fill=False)
                    ha = stage_ag(t, ag_a, 384, 0, "ha")
                    emit_opj(2)
                    agt = load_agt(staged_map[t - 1], t - 1, ways=4, bufs=1)
                    for st in range(4):
                        opq.append((t - 1, st, agt))
                    emit_opj(2)
                    # load the first piece's gather BEFORE the second
                    # piece's stores enter the DMA queues (it only waits on
                    # the first collective, completing mid-second-piece)
                    hat = load_agt(ha, t, ways=4, bufs=1)
                    ag_b = att_pass(t, 384, 128, "b", "hb", fill=False)
                    hb = stage_ag(t, ag_b, 128, 384, "hb")
                    emit_opj(2)
                    opq += [(t, 0, hat), (t, 1, hat), (t, 2, hat)]
                    emit_opj(3)
                    hbt = load_agt(hb, t, ways=4, bufs=1)
                    opq.append((t, 3, hbt))
            emit_opj(len(opq))
    nc.compile()
    return nc


def _host_prep(x, wq, wk, wv, wo):
    """Build per-core input maps (host-side transposes + bf16 casts)."""
    bf = ml_dtypes.bfloat16
    # rope tables in the transposed [d, s] layout
    inv = 1.0 / (10000.0 ** (np.arange(0, D, 2, dtype=np.float64) / D))  # [64]
    ang = np.outer(np.arange(S, dtype=np.float64), inv)                  # [S, 64]
    cos = np.cos(ang).T        # [64, S]
    sin = np.sin(ang).T        # [64, S]
    cosb = np.repeat(cos, 2, axis=0).astype(np.float32)                  # [128, S]
    sinb = np.repeat(sin, 2, axis=0).astype(np.float32)
    sinb[0::2, :] *= -1.0      # even d rows: -sin ; odd rows: +sin

    ki = np.arange(128)[:, None]
    qj = np.arange(512)[None, :]
    msk_m = np.stack([(j * 128 + ki <= qj).astype(np.float32) for j in range(4)])

    xT_b = [np.ascontiguousarray(x[b].T).astype(bf) for b in range(B)]
    cosb, sinb = cosb.astype(bf), sinb.astype(bf)
    msk_m = msk_m.astype(bf)

    in_maps = []
    for c in range(NCORES):
        hrows = slice((c % 4) * HD, (c % 4 + 1) * HD)     # this core's 4 heads
        ocols = slice((c % 4) * OSL, (c % 4 + 1) * OSL)   # its output columns
        in_maps.append({
            "xT": xT_b[c // 4],
            "wqT": np.ascontiguousarray(wq[hrows].T).astype(bf),
            "wkT": np.ascontiguousarray(wk[hrows].T).astype(bf),
            "wvT": np.ascontiguousarray(wv[hrows].T).astype(bf),
            "woT": np.ascontiguousarray(wo[ocols, :].T).astype(bf),
            "cosb": cosb,
            "sinb": sinb,
            "msk": msk_m,
        })
    return in_maps


def kernel(x, wq, wk, wv, wo):
    global LAST_RESULT, _CACHED_NC
    if _CACHED_NC is None:
        _CACHED_NC = _build()
    nc = _CACHED_NC
    in_maps = _host_prep(x, wq, wk, wv, wo)
    res = run_bass_kernel_spmd(nc, in_maps, core_ids=list(range(NCORES)))
    LAST_RESULT = res
    out = np.empty((B, S, DIM), np.float32)
    for c in range(NCORES):
        bb = c // 4
        csl = slice((c % 4) * OSL, (c % 4 + 1) * OSL)
        out[bb, :, csl] = res.results[c]["out"]
    return out
